# revision 25
# baseline (speedup 1.0000x reference)
"""Trainium2 Bass kernel for a binarized (1w/1a) BasicBlock — fp8 DoubleRow.

    a1 = sign(x);  y1 = BN(conv3x3(a1, binarize(w1))) + x;  x1 = maxout(y1)
    a2 = sign(x1); y2 = BN(conv3x3(a2, binarize(w2))) + x1; out = maxout(y2)

Data-parallel over batch (4 samples/core, 8 cores); exact binary math:
activations are +-1 (fp8e4, exact), weights are sign(+-1) fp8; each conv is
9 DoubleRow matmuls per (chunk, cout-block), contracting all 256 input
channels at once over contiguous padded-row runs (pad columns land in
unused psum columns).  conv_true = alpha_a*alpha[o]*(BB + q[o]*S1) with
q = beta/alpha; S1 (3x3 box of the channel sum) comes from 3 more DoubleRow
ones-matmuls (folding the kh taps) + 2 shifted adds.  The per-channel scale
folds into BN exactly by scaling BN_EPS per channel.  Batch-stat BN sends
per-core (sum, sumsq) pairs through one AllGather per round; every core
reduces the 8 contributions locally.

Everything is pipelined per sample: x-DMA -> sign -> S1 -> main matmuls
(round 1), and apply(round1) -> sign -> S1 -> main (round 2), so the only
serial points are the two AllGathers (the first of which is gated by the
NRT init barrier anyway).

Maxout is sign-based: out = t * (sign(t)*(p-n)/2 + (p+n)/2), reusing the
sign values the next conv needs anyway.
"""

import numpy as np
import ml_dtypes

import concourse.bass as bass
import concourse.bacc as bacc
import concourse.mybir as mybir
import concourse.tile as tile

N_CORES = 8
B, C, H, W = 32, 256, 28, 28
BPC = B // N_CORES            # samples per core
NBLK = 2                      # channel blocks of 128
HPAD, WPAD = 30, 30           # padded image in SBUF
PIX = H * W                   # 784
PPIX = HPAD * WPAD            # 900
SPP = 912                     # padded plane stride (16B-aligned > PPIX)
NCHUNK = 2 * BPC              # 8 chunks of (sample, half-image)
HHALF = H // 2                # 14
CHUNK = HHALF * W             # 392 dense output elems per chunk
RUN = HHALF * WPAD            # 420: rhs run length / psum width per chunk
S1RUN = SPP // 2              # 456: ones-matmul run (half plane)
BN_EPS = 1e-5
NPRM = 24
GUARD = 16                    # fp8 guard elems around activation tiles
NTOT = float(N_CORES * NCHUNK * CHUNK)   # global BN count 25088
F32 = mybir.dt.float32
FP8 = mybir.dt.float8e4
AF = mybir.ActivationFunctionType
ALU = mybir.AluOpType
DR = mybir.MatmulPerfMode.DoubleRow


def _evac(nc, sc, ps, s1, sums, sumsqs, cv, prm, pcol, ci, oblk):
    """z = q[o]*S1 + BB from PSUM (strided: skip pad cols).  Stats come for
    free: the STT accumulates sum(z) on DVE; a Square pass on the otherwise
    idle ScalarE accumulates sum(z^2)."""
    psv = ps[:].rearrange("p (h w) -> p h w", h=HHALF)[:, :, 1:1 + W]
    s1v = s1[:].rearrange("p (h w) -> p h w", h=H)[
        :, (ci % 2) * HHALF:(ci % 2) * HHALF + HHALF, :]
    cvc = cv[oblk][:, ci * CHUNK:(ci + 1) * CHUNK]
    nc.vector.scalar_tensor_tensor(
        cvc.rearrange("p (h w) -> p h w", h=HHALF), s1v,
        prm[:, pcol['q'] + oblk:pcol['q'] + oblk + 1], psv,
        op0=ALU.mult, op1=ALU.add,
        accum_out=sums[:, oblk * NCHUNK + ci:oblk * NCHUNK + ci + 1])
    sqj = sc.tile([128, CHUNK], F32, tag="sqj", name="sqj", bufs=2)
    nc.scalar.activation(
        sqj[:], cvc, AF.Square,
        accum_out=sumsqs[:, oblk * NCHUNK + ci:oblk * NCHUNK + ci + 1])


def _s1_sample(nc, sc, psum, ones3, rhs420, rnd):
    """3x3 box of the channel sum for one sample -> s1 tile [128, H*W].
    The kh taps fold into PSUM accumulation of 3 shifted ones-matmuls per
    half; the kw taps are 2 shifted adds (GpSimd + DVE)."""
    hs = sc.tile([128, 2 * RUN], F32, tag="hs", name="hs", bufs=2)
    for half in range(2):
        h0 = half * HHALF
        ps2 = psum.tile([128, RUN], F32, tag="ps2", name=f"ps2_{rnd}",
                        bufs=2)
        for kh in range(3):
            nc.tensor.matmul(ps2[:], ones3, rhs420((h0 + kh) * WPAD),
                             start=(kh == 0), stop=(kh == 2), perf_mode=DR)
        nc.scalar.copy(hs[:, half * RUN:half * RUN + RUN], ps2[:])
    hsv = hs[:].rearrange("p (h w) -> p h w", h=H)
    w3 = sc.tile([128, H * W], F32, tag="w3", name="w3", bufs=2)
    w3v = w3[:].rearrange("p (h w) -> p h w", h=H)
    nc.gpsimd.tensor_add(w3v, hsv[:, :, 0:W], hsv[:, :, 1:1 + W])
    s1 = sc.tile([128, H * W], F32, tag="s1", name="s1", bufs=4)
    s1v = s1[:].rearrange("p (h w) -> p h w", h=H)
    nc.vector.tensor_add(s1v, w3v, hsv[:, :, 2:2 + W])
    return s1


def _main_sample(nc, sc, psum, rhs_ap, wv, b, rnd, s1, sums, sumsqs, cv,
                 prm, pcol, oblks=tuple(range(NBLK))):
    """Main conv matmuls + evac for sample b over the given cout blocks."""
    for half in range(2):
        h0 = half * HHALF
        ci = 2 * b + half
        for oblk in oblks:
            ps = psum.tile([128, RUN], F32, tag="ps", name=f"ps{rnd}",
                           bufs=6)
            for k9 in range(9):
                kh, kw = k9 // 3, k9 % 3
                nc.tensor.matmul(
                    ps[:], wv[:, k9, :, oblk * 128:(oblk + 1) * 128],
                    rhs_ap((h0 + kh) * WPAD + kw - 1),
                    start=(k9 == 0), stop=(k9 == 8), perf_mode=DR)
            _evac(nc, sc, ps, s1, sums, sumsqs, cv, prm, pcol, ci, oblk)


def _stats(nc, pools, rnd, sums, sumsqs, prm, pcol):
    """(sum, sumsq) per chunk -> per-core [128, (blk j)] -> one AllGather ->
    global scale/shift (128, NBLK) tiles indexed by oblk."""
    sbuf, psum, sc, dram = pools
    tr = sbuf.tile([128, 2 * NBLK], F32, name=f"tr{rnd}")
    for oblk in range(NBLK):
        nc.vector.reduce_sum(tr[:, oblk * 2:oblk * 2 + 1],
                             sums[:, oblk * NCHUNK:(oblk + 1) * NCHUNK],
                             axis=mybir.AxisListType.X)
        nc.vector.reduce_sum(tr[:, oblk * 2 + 1:oblk * 2 + 2],
                             sumsqs[:, oblk * NCHUNK:(oblk + 1) * NCHUNK],
                             axis=mybir.AxisListType.X)
    b_d = dram.tile([128, 2 * NBLK], F32, name=f"bd{rnd}")
    g_d = dram.tile([N_CORES * 128, 2 * NBLK], F32, name=f"gd{rnd}")
    nc.sync.dma_start(b_d[:], tr[:])
    nc.gpsimd.collective_compute(
        "AllGather", ALU.bypass,
        replica_groups=[list(range(N_CORES))],
        ins=[b_d.opt()], outs=[g_d.opt()])
    gst = sbuf.tile([128, N_CORES * 2 * NBLK], F32, name=f"gst{rnd}")
    nc.sync.dma_start(
        gst[:].rearrange("p (r j) -> p r j", r=N_CORES),
        g_d[:].rearrange("(r p) j -> p r j", r=N_CORES))
    gv = gst[:].rearrange("p (r j) -> p r j", r=N_CORES)

    mean = sbuf.tile([128, NBLK], F32, name=f"mean{rnd}")
    qn = sbuf.tile([128, NBLK], F32, name=f"qn{rnd}")
    msq = sbuf.tile([128, NBLK], F32, name=f"msq{rnd}")
    var = sbuf.tile([128, NBLK], F32, name=f"var{rnd}")
    sd = sbuf.tile([128, NBLK], F32, name=f"sd{rnd}")
    inv = sbuf.tile([128, NBLK], F32, name=f"inv{rnd}")
    scale = sbuf.tile([128, NBLK], F32, name=f"scale{rnd}")
    tmp = sbuf.tile([128, NBLK], F32, name=f"tmp{rnd}")
    shift = sbuf.tile([128, NBLK], F32, name=f"shift{rnd}")
    for oblk in range(NBLK):
        nc.vector.reduce_sum(mean[:, oblk:oblk + 1], gv[:, :, oblk * 2],
                             axis=mybir.AxisListType.X)
        nc.vector.reduce_sum(qn[:, oblk:oblk + 1], gv[:, :, oblk * 2 + 1],
                             axis=mybir.AxisListType.X)
    nc.vector.tensor_scalar_mul(mean[:], mean[:], 1.0 / NTOT)
    nc.vector.tensor_scalar_mul(qn[:], qn[:], 1.0 / NTOT)
    nc.vector.tensor_mul(msq[:], mean[:], mean[:])
    nc.vector.tensor_sub(var[:], qn[:], msq[:])
    for oblk in range(NBLK):
        nc.scalar.activation(sd[:, oblk:oblk + 1], var[:, oblk:oblk + 1],
                             AF.Sqrt,
                             bias=prm[:, pcol['eps'] + oblk:
                                      pcol['eps'] + oblk + 1],
                             scale=1.0)
    nc.vector.reciprocal(inv[:], sd[:])
    nc.vector.tensor_mul(scale[:], inv[:],
                         prm[:, pcol['g']:pcol['g'] + NBLK])
    nc.vector.tensor_mul(tmp[:], mean[:], scale[:])
    nc.vector.tensor_sub(shift[:], prm[:, pcol['b']:pcol['b'] + NBLK],
                         tmp[:])
    return scale, shift


def _stats_ob_start(nc, pools, rnd, oblk, sums, sumsqs):
    """Reduce this block's (sum, sumsq) and launch its AllGather; returns
    the gathered-SBUF tile (filled once the collective lands)."""
    sbuf, psum, sc, dram = pools
    tr = sbuf.tile([128, 2], F32, name=f"tr{rnd}o{oblk}")
    nc.vector.reduce_sum(tr[:, 0:1],
                         sums[:, oblk * NCHUNK:(oblk + 1) * NCHUNK],
                         axis=mybir.AxisListType.X)
    nc.vector.reduce_sum(tr[:, 1:2],
                         sumsqs[:, oblk * NCHUNK:(oblk + 1) * NCHUNK],
                         axis=mybir.AxisListType.X)
    b_d = dram.tile([128, 2], F32, name=f"bd{rnd}o{oblk}")
    g_d = dram.tile([N_CORES * 128, 2], F32, name=f"gd{rnd}o{oblk}")
    nc.sync.dma_start(b_d[:], tr[:])
    nc.gpsimd.collective_compute(
        "AllGather", ALU.bypass,
        replica_groups=[list(range(N_CORES))],
        ins=[b_d.opt()], outs=[g_d.opt()])
    gst = sbuf.tile([128, N_CORES * 2], F32, name=f"gst{rnd}o{oblk}")
    nc.gpsimd.dma_start(
        gst[:].rearrange("p (r j) -> p r j", r=N_CORES),
        g_d[:].rearrange("(r p) j -> p r j", r=N_CORES))
    return gst


def _stats_ob_finish(nc, pools, rnd, oblk, gst, prm, pcol):
    """Gathered 8x(sum,sumsq) -> scale/shift [128, 1] for this block."""
    sbuf, psum, sc, dram = pools
    gv = gst[:].rearrange("p (r j) -> p r j", r=N_CORES)
    mean = sbuf.tile([128, 1], F32, name=f"mean{rnd}o{oblk}")
    qn = sbuf.tile([128, 1], F32, name=f"qn{rnd}o{oblk}")
    msq = sbuf.tile([128, 1], F32, name=f"msq{rnd}o{oblk}")
    var = sbuf.tile([128, 1], F32, name=f"var{rnd}o{oblk}")
    sd = sbuf.tile([128, 1], F32, name=f"sd{rnd}o{oblk}")
    inv = sbuf.tile([128, 1], F32, name=f"inv{rnd}o{oblk}")
    scale = sbuf.tile([128, 1], F32, name=f"scale{rnd}o{oblk}")
    tmp = sbuf.tile([128, 1], F32, name=f"tmp{rnd}o{oblk}")
    shift = sbuf.tile([128, 1], F32, name=f"shift{rnd}o{oblk}")
    nc.vector.reduce_sum(mean[:], gv[:, :, 0], axis=mybir.AxisListType.X)
    nc.vector.reduce_sum(qn[:], gv[:, :, 1], axis=mybir.AxisListType.X)
    nc.vector.tensor_scalar_mul(mean[:], mean[:], 1.0 / NTOT)
    nc.vector.tensor_scalar_mul(qn[:], qn[:], 1.0 / NTOT)
    nc.vector.tensor_mul(msq[:], mean[:], mean[:])
    nc.vector.tensor_sub(var[:], qn[:], msq[:])
    nc.scalar.activation(sd[:], var[:], AF.Sqrt,
                         bias=prm[:, pcol['eps'] + oblk:
                                  pcol['eps'] + oblk + 1],
                         scale=1.0)
    nc.vector.reciprocal(inv[:], sd[:])
    nc.vector.tensor_mul(scale[:], inv[:],
                         prm[:, pcol['g'] + oblk:pcol['g'] + oblk + 1])
    nc.vector.tensor_mul(tmp[:], mean[:], scale[:])
    nc.vector.tensor_sub(shift[:], prm[:, pcol['b'] + oblk:
                                       pcol['b'] + oblk + 1], tmp[:])
    return scale, shift


def _apply_front(nc, sc, b, oblk, cv, xres, scale_ap, shift_ap, prm, pcol,
                 a2v=None):
    """BN-apply + residual + maxout coef for one (sample, block) span.
    Writes sign into a2v (next conv's fp8 input); returns (t, coef)."""
    cvs = cv[oblk][:, b * PIX:(b + 1) * PIX]
    xrs = xres[oblk][:, b * PIX:(b + 1) * PIX]
    u = sc.tile([128, PIX], F32, tag="u", name="u", bufs=2)
    nc.vector.tensor_scalar(u[:], cvs, scale_ap, shift_ap,
                            op0=ALU.mult, op1=ALU.add)
    t = sc.tile([128, PIX], F32, tag="t", name="t", bufs=4)
    nc.vector.tensor_add(t[:], u[:], xrs)
    tv = t[:].rearrange("p (h w) -> p h w", h=H)
    coef = sc.tile([128, PIX], F32, tag="coef", name="coef", bufs=4)
    if a2v is not None:
        # sign -> next conv's fp8 input (padded); coef read from it
        sg = a2v[b][:, oblk, 1:1 + H, 1:1 + W]
        nc.scalar.activation(sg, tv, AF.Sign)
        nc.scalar.activation(
            coef[:].rearrange("p (h w) -> p h w", h=H),
            sg, AF.Identity,
            bias=prm[:, pcol['hs'] + oblk:pcol['hs'] + oblk + 1],
            scale=prm[:, pcol['hp'] + oblk:pcol['hp'] + oblk + 1])
    else:
        sgt = sc.tile([128, PIX], F32, tag="sg", name="sg", bufs=2)
        nc.scalar.activation(sgt[:], t[:], AF.Sign)
        nc.scalar.activation(
            coef[:], sgt[:], AF.Identity,
            bias=prm[:, pcol['hs'] + oblk:pcol['hs'] + oblk + 1],
            scale=prm[:, pcol['hp'] + oblk:pcol['hp'] + oblk + 1])
    return t, coef


def _apply_back(nc, sc, b, oblk, t, coef, xres, ov=None, eng=None):
    """maxout multiply: in-place into xres (mid rounds) or out DMA."""
    e = eng if eng is not None else nc.vector
    xrs = xres[oblk][:, b * PIX:(b + 1) * PIX]
    if ov is None:
        e.tensor_mul(xrs, t[:], coef[:])
    else:
        och = sc.tile([128, PIX], F32, tag="och", name="och", bufs=4)
        e.tensor_mul(och[:], t[:], coef[:])
        q = nc.sync if (2 * b + oblk) % 2 == 0 else nc.scalar
        q.dma_start(ov[oblk * 128:oblk * 128 + 128, b], och[:])


def _apply_all(nc, sc, cv, xres, scale, shift, prm, pcol, a2v=None, ov=None):
    """All 8 apply spans with the maxout mul deferred one span, so the
    vector engine never blocks on the scalar sign->coef chain; muls
    alternate between DVE and GpSimd."""
    spans = [(b, oblk) for b in range(BPC) for oblk in range(NBLK)]
    pend = []
    n = 0
    for b, oblk in spans:
        t, coef = _apply_front(nc, sc, b, oblk, cv, xres, scale, shift,
                               prm, pcol, a2v=a2v)
        pend.append((b, oblk, t, coef))
        if len(pend) > 1:
            pb, po, pt, pc = pend.pop(0)
            _apply_back(nc, sc, pb, po, pt, pc, xres, ov=ov,
                        eng=nc.gpsimd if n % 2 else nc.vector)
            n += 1
    for pb, po, pt, pc in pend:
        _apply_back(nc, sc, pb, po, pt, pc, xres, ov=ov,
                    eng=nc.gpsimd if n % 2 else nc.vector)
        n += 1


def build():
    nc = bacc.Bacc("TRN2", target_bir_lowering=False, debug=False,
                   enable_asserts=True, num_devices=N_CORES)
    x_d = nc.dram_tensor("x", [BPC, C, H, W], F32, kind="ExternalInput")
    w1_d = nc.dram_tensor("w1t", [9, NBLK, 128, 256], FP8,
                          kind="ExternalInput")
    w2_d = nc.dram_tensor("w2t", [9, NBLK, 128, 256], FP8,
                          kind="ExternalInput")
    prm_d = nc.dram_tensor("prm", [128, NPRM], F32, kind="ExternalInput")
    out_d = nc.dram_tensor("out", [BPC, C, H, W], F32, kind="ExternalOutput")

    with tile.TileContext(nc) as tc:
        with (
            tc.tile_pool(name="sbuf", bufs=1) as sbuf,
            tc.tile_pool(name="psum", bufs=6, space="PSUM") as psum,
            tc.tile_pool(name="sc", bufs=2) as sc,
            tc.tile_pool(name="dram", bufs=1, space="DRAM") as dram,
        ):
            pools = (sbuf, psum, sc, dram)
            w1sb = sbuf.tile([128, 9 * NBLK * 256], FP8, name="w1sb")
            w2sb = sbuf.tile([128, 9 * NBLK * 256], FP8, name="w2sb")
            prm = sbuf.tile([128, NPRM], F32, name="prm")
            onesb = sbuf.tile([128, 256], FP8, name="onesb")
            xres = [sbuf.tile([128, BPC * PIX], F32, name=f"xres{i}")
                    for i in range(NBLK)]
            # per-sample padded activation tiles (two K-half planes each) so
            # subtile deps stay exact: conv(b) only waits on sign(b)
            ATW = GUARD + 2 * SPP + GUARD
            a1s = [sbuf.tile([128, ATW], FP8, name=f"a1s{b}")
                   for b in range(BPC)]
            a2s = [sbuf.tile([128, ATW], FP8, name=f"a2s{b}")
                   for b in range(BPC)]
            cv = [sbuf.tile([128, BPC * PIX], F32, name=f"cv{i}")
                  for i in range(NBLK)]

            # warmup collective, triggered first: absorbs the per-launch CC
            # spin-up behind the NRT init barrier so the stats AllGather
            # runs warm and starts promptly
            wu_i = dram.tile([1, 16], F32, name="wu_i")
            wu_o = dram.tile([N_CORES, 16], F32, name="wu_o")
            wu_s = sbuf.tile([1, 16], F32, name="wu_s")
            nc.gpsimd.memset(wu_s[:], 0.0)
            nc.gpsimd.dma_start(wu_i[:], wu_s[:])
            nc.gpsimd.collective_compute(
                "AllGather", ALU.bypass,
                replica_groups=[list(range(N_CORES))],
                ins=[wu_i.opt()], outs=[wu_o.opt()])
            nc.vector.memset(onesb[:], 1.0)
            for b in range(BPC):
                nc.vector.memset(a1s[b][:].bitcast(mybir.dt.uint32), 0)
                nc.gpsimd.memset(a2s[b][:].bitcast(mybir.dt.uint32), 0)
            xv = x_d[:].rearrange("b c h w -> c b (h w)")
            w1v = w1sb[:].rearrange("p (k i o) -> p k i o", k=9, i=NBLK)
            w1dv = w1_d[:].rearrange("k i p o -> p k i o")
            # queue layout: weights + prm first (small), then x in sample
            # order striped over both HWDGE queues, then w2
            nc.sync.dma_start(prm[:], prm_d[:])
            nc.scalar.dma_start(w1v[:, 0:3], w1dv[:, 0:3])
            nc.gpsimd.dma_start(w1v[:, 3:6], w1dv[:, 3:6])
            nc.sync.dma_start(w1v[:, 6:9], w1dv[:, 6:9])
            for b in range(BPC):
                nc.scalar.dma_start(xres[0][:, b * PIX:(b + 1) * PIX],
                                    xv[0:128, b])
                nc.sync.dma_start(xres[1][:, b * PIX:(b + 1) * PIX],
                                  xv[128:256, b])
            nc.sync.dma_start(
                w2sb[:].rearrange("p (k i o) -> p k i o", k=9, i=NBLK),
                w2_d[:].rearrange("k i p o -> p k i o"))

            def mk_rhs(atile, b, n):
                trun = atile[b][:, 0:2 * SPP].rearrange(
                    "p (i n) -> p i n", i=2)[:, :, 0:n]

                def rhs_ap(off):
                    return bass.AP(trun.tensor, GUARD + off, trun.ap)
                return rhs_ap

            rhs1 = [mk_rhs(a1s, b, RUN) for b in range(BPC)]
            rhs2 = [mk_rhs(a2s, b, RUN) for b in range(BPC)]
            ones3 = onesb[:].rearrange("p (i o) -> p i o", i=2)
            wv1 = w1sb[:].rearrange("p (k i o) -> p k i o", k=9, i=NBLK)
            wv2 = w2sb[:].rearrange("p (k i o) -> p k i o", k=9, i=NBLK)

            def aview(atile, b):
                return atile[b][:, GUARD:GUARD + 2 * SPP].rearrange(
                    "p (i n) -> p i n", i=2)[:, :, 0:PPIX].rearrange(
                    "p i (h w) -> p i h w", h=HPAD)

            a1v = [aview(a1s, b) for b in range(BPC)]
            a2v = [aview(a2s, b) for b in range(BPC)]
            xrvs = [xres[i][:].rearrange("p (b h w) -> p b h w", b=BPC, h=H)
                    for i in range(NBLK)]
            ov = out_d[:].rearrange("b c h w -> c b (h w)")

            pcol1 = {'g': 0, 'b': 2, 'hp': 4, 'hs': 6, 'eps': 16, 'q': 20}
            pcol2 = {'g': 8, 'b': 10, 'hp': 12, 'hs': 14, 'eps': 18, 'q': 22}

            def signs(b):
                for i in range(NBLK):
                    nc.scalar.activation(a1v[b][:, i, 1:1 + H, 1:1 + W],
                                         xrvs[i][:, b], AF.Sign)

            # ---- round 1: per-sample sign -> S1 -> main conv; the next
            # sample's sign is issued between S1 and main so ScalarE runs it
            # before this sample's evac Squares ----
            sums1 = sbuf.tile([128, NCHUNK * NBLK], F32, name="sums1")
            sumsqs1 = sbuf.tile([128, NCHUNK * NBLK], F32, name="sumsqs1")
            signs(0)
            for b in range(BPC):
                s1 = _s1_sample(nc, sc, psum, ones3, rhs1[b], 1)
                if b + 1 < BPC:
                    signs(b + 1)
                _main_sample(nc, sc, psum, rhs1[b], wv1, b, 1, s1,
                             sums1, sumsqs1, cv, prm, pcol1)
            scale1, shift1 = _stats(nc, pools, 1, sums1, sumsqs1, prm, pcol1)

            # ---- apply round 1 interleaved per sample with round 2 conv;
            # the maxout muls go to GpSimd and are deferred one span ----
            sums2 = sbuf.tile([128, NCHUNK * NBLK], F32, name="sums2")
            sumsqs2 = sbuf.tile([128, NCHUNK * NBLK], F32, name="sumsqs2")
            pend = []
            s1s = []
            for b in range(BPC):
                for oblk in range(NBLK):
                    t, coef = _apply_front(
                        nc, sc, b, oblk, cv, xres,
                        scale1[:, oblk:oblk + 1], shift1[:, oblk:oblk + 1],
                        prm, pcol1, a2v=a2v)
                    pend.append((b, oblk, t, coef))
                    if len(pend) > 1:
                        pb, po, pt, pc = pend.pop(0)
                        _apply_back(nc, sc, pb, po, pt, pc, xres,
                                    eng=nc.gpsimd)
                s1 = _s1_sample(nc, sc, psum, ones3, rhs2[b], 2)
                s1s.append(s1)
                _main_sample(nc, sc, psum, rhs2[b], wv2, b, 2, s1,
                             sums2, sumsqs2, cv, prm, pcol2, oblks=(0,))
            for pb, po, pt, pc in pend:
                _apply_back(nc, sc, pb, po, pt, pc, xres, eng=nc.gpsimd)
            # block 0's AllGather launches at conv2's midpoint and hides
            # under block 1's matmuls; block 1's AllGather hides under
            # block 0's final apply
            gst_a = _stats_ob_start(nc, pools, 2, 0, sums2, sumsqs2)
            for b in range(BPC):
                _main_sample(nc, sc, psum, rhs2[b], wv2, b, 2, s1s[b],
                             sums2, sumsqs2, cv, prm, pcol2, oblks=(1,))
            gst_b = _stats_ob_start(nc, pools, 2, 1, sums2, sumsqs2)
            sc_a, sh_a = _stats_ob_finish(nc, pools, 2, 0, gst_a, prm, pcol2)

            # ---- final apply + output DMA, per block ----
            def final_apply(ob, sca, sha):
                p2 = []
                for b in range(BPC):
                    t, coef = _apply_front(nc, sc, b, ob, cv, xres,
                                           sca[:, 0:1], sha[:, 0:1],
                                           prm, pcol2)
                    p2.append((b, ob, t, coef))
                    if len(p2) > 1:
                        pb, po, pt, pc = p2.pop(0)
                        _apply_back(nc, sc, pb, po, pt, pc, xres, ov=ov)
                for pb, po, pt, pc in p2:
                    _apply_back(nc, sc, pb, po, pt, pc, xres, ov=ov)

            final_apply(0, sc_a, sh_a)
            sc_b, sh_b = _stats_ob_finish(nc, pools, 2, 1, gst_b, prm, pcol2)
            final_apply(1, sc_b, sh_b)

    nc.compile()
    return nc


def _prep_weight(w):
    """(O,I,3,3) fp32 -> sign lhsT (9, iblk, 128, 256) fp8 (+-1, exact),
    plus per-output-channel alpha, beta (float64)."""
    w = w.astype(np.float64)
    beta = w.mean(axis=(1, 2, 3))
    alpha = np.sqrt(((w - beta[:, None, None, None]) ** 2)
                    .mean(axis=(1, 2, 3)))
    s = np.sign(w - beta[:, None, None, None]).astype(np.float32)
    wt = s.transpose(2, 3, 1, 0).reshape(9, C, C)   # (k9, i, o)
    wt = wt.reshape(9, NBLK, 128, C)                # (k9, iblk, i, o)
    return wt.astype(ml_dtypes.float8_e4m3), alpha, beta


def make_in_maps(inputs):
    x = np.asarray(inputs['x'], np.float32)
    aa1 = float(np.asarray(inputs['alpha_a1']).reshape(-1)[0])
    aa2 = float(np.asarray(inputs['alpha_a2']).reshape(-1)[0])
    w1t, al1, be1 = _prep_weight(np.asarray(inputs['w1'], np.float32))
    w2t, al2, be2 = _prep_weight(np.asarray(inputs['w2'], np.float32))
    prm = np.zeros((128, NPRM), np.float32)
    f1 = 1.0 / (aa1 * al1)      # z scale relative to the true conv output
    f2 = 1.0 / (aa2 * al2)
    p1 = np.asarray(inputs['pos1'], np.float64)
    n1 = np.asarray(inputs['neg1'], np.float64)
    p2 = np.asarray(inputs['pos2'], np.float64)
    n2 = np.asarray(inputs['neg2'], np.float64)
    cols = ((0, np.asarray(inputs['g1'], np.float64)),
            (2, np.asarray(inputs['b1'], np.float64)),
            (4, (p1 - n1) / 2),
            (6, (p1 + n1) / 2),
            (8, np.asarray(inputs['g2'], np.float64)),
            (10, np.asarray(inputs['b2'], np.float64)),
            (12, (p2 - n2) / 2),
            (14, (p2 + n2) / 2),
            (16, BN_EPS * f1 * f1),
            (18, BN_EPS * f2 * f2),
            (20, be1 / al1),
            (22, be2 / al2))
    for base, arr in cols:
        prm[:, base] = arr[:128]
        prm[:, base + 1] = arr[128:]
    in_maps = []
    for c in range(N_CORES):
        in_maps.append({
            'x': np.ascontiguousarray(x[c * BPC:(c + 1) * BPC]),
            'w1t': w1t, 'w2t': w2t, 'prm': prm,
        })
    return in_maps


_CACHE = {}


def kernel(**inputs):
    in_maps = make_in_maps(inputs)
    if 'run' not in _CACHE:
        nc = build()
        _CACHE['nc'] = nc
        _CACHE['run'] = _make_runner(nc)
    outs = _CACHE['run'](in_maps)
    return np.concatenate([outs[c] for c in range(N_CORES)], axis=0)


def _make_runner(nc):
    """Build a cached PJRT executable (same path run_bass_kernel_spmd takes
    under axon, via bass2jax) so repeat calls don't re-trace."""
    import jax
    import jax.numpy as jnp
    from jax.sharding import Mesh, PartitionSpec
    from jax.experimental.shard_map import shard_map
    from concourse import bass2jax

    bass2jax.install_neuronx_cc_hook()
    partition_name = (nc.partition_id_tensor.name
                      if nc.partition_id_tensor else None)
    in_names = []
    out_names = []
    out_avals = []
    for alloc in nc.m.functions[0].allocations:
        if not isinstance(alloc, mybir.MemoryLocationSet):
            continue
        name = alloc.memorylocations[0].name
        if alloc.kind == "ExternalInput":
            if name != partition_name:
                in_names.append(name)
        elif alloc.kind == "ExternalOutput":
            shape = tuple(alloc.tensor_shape)
            dtype = mybir.dt.np(alloc.dtype)
            out_names.append(name)
            out_avals.append(jax.core.ShapedArray(shape, dtype))
    n_params = len(in_names)
    all_names = in_names + out_names
    if partition_name is not None:
        all_names = all_names + [partition_name]

    def _body(*args):
        operands = list(args)
        if partition_name is not None:
            operands.append(bass2jax.partition_id_tensor())
        outs = bass2jax._bass_exec_p.bind(
            *operands,
            out_avals=tuple(out_avals),
            in_names=tuple(all_names),
            out_names=tuple(out_names),
            lowering_input_output_aliases=(),
            sim_require_finite=True,
            sim_require_nnan=True,
            nc=nc,
        )
        return tuple(outs)

    devices = jax.devices()[:N_CORES]
    mesh = Mesh(np.asarray(devices), ("core",))
    n_outs = len(out_names)
    sharded = jax.jit(
        shard_map(_body, mesh=mesh,
                  in_specs=(PartitionSpec("core"),) * (n_params + n_outs),
                  out_specs=(PartitionSpec("core"),) * n_outs,
                  check_rep=False),
        donate_argnums=tuple(range(n_params, n_params + n_outs)),
        keep_unused=True,
    )
    sharded_nodonate = jax.jit(
        shard_map(_body, mesh=mesh,
                  in_specs=(PartitionSpec("core"),) * (n_params + n_outs),
                  out_specs=(PartitionSpec("core"),) * n_outs,
                  check_rep=False),
        keep_unused=True,
    )

    def run(in_maps):
        concat_in = [
            np.concatenate([np.asarray(in_maps[c][n]) for c in range(N_CORES)],
                           axis=0)
            for n in in_names
        ]
        concat_zeros = [
            np.zeros((N_CORES * a.shape[0], *a.shape[1:]), a.dtype)
            for a in out_avals
        ]
        out_arrs = sharded(*concat_in, *concat_zeros)
        i = out_names.index("out")
        full = np.asarray(out_arrs[i]).reshape(N_CORES, *out_avals[i].shape)
        return [full[c] for c in range(N_CORES)]

    def stage(in_maps):
        """device_put inputs once; return a dispatch closure for timing."""
        from jax.sharding import NamedSharding
        sh = NamedSharding(mesh, PartitionSpec("core"))
        concat_in = [
            jax.device_put(np.concatenate(
                [np.asarray(in_maps[c][n]) for c in range(N_CORES)], axis=0), sh)
            for n in in_names
        ]
        concat_zeros = [
            jax.device_put(
                np.zeros((N_CORES * a.shape[0], *a.shape[1:]), a.dtype), sh)
            for a in out_avals
        ]

        def dispatch():
            return sharded_nodonate(*concat_in, *concat_zeros)

        return dispatch

    run.stage = stage
    return run


# revision 26
# speedup vs baseline: 1.0718x; 1.0718x over previous
"""Trainium2 Bass kernel for a binarized (1w/1a) BasicBlock — fp8 DoubleRow.

    a1 = sign(x);  y1 = BN(conv3x3(a1, binarize(w1))) + x;  x1 = maxout(y1)
    a2 = sign(x1); y2 = BN(conv3x3(a2, binarize(w2))) + x1; out = maxout(y2)

Data-parallel over batch (4 samples/core, 8 cores); exact binary math:
activations are +-1 (fp8e4, exact), weights are sign(+-1) fp8; each conv is
9 DoubleRow matmuls per (chunk, cout-block), contracting all 256 input
channels at once over contiguous padded-row runs (pad columns land in
unused psum columns).  conv_true = alpha_a*alpha[o]*(BB + q[o]*S1) with
q = beta/alpha; S1 (3x3 box of the channel sum) comes from 3 more DoubleRow
ones-matmuls (folding the kh taps) + 2 shifted adds.  The per-channel scale
folds into BN exactly by scaling BN_EPS per channel.  Batch-stat BN sends
per-core (sum, sumsq) pairs through one AllGather per round; every core
reduces the 8 contributions locally.

Everything is pipelined per sample: x-DMA -> sign -> S1 -> main matmuls
(round 1), and apply(round1) -> sign -> S1 -> main (round 2), so the only
serial points are the two AllGathers (the first of which is gated by the
NRT init barrier anyway).

Maxout is sign-based: out = t * (sign(t)*(p-n)/2 + (p+n)/2), reusing the
sign values the next conv needs anyway.
"""

import numpy as np
import ml_dtypes

import concourse.bass as bass
import concourse.bacc as bacc
import concourse.mybir as mybir
import concourse.tile as tile

N_CORES = 8
B, C, H, W = 32, 256, 28, 28
BPC = B // N_CORES            # samples per core
NBLK = 2                      # channel blocks of 128
HPAD, WPAD = 30, 30           # padded image in SBUF
PIX = H * W                   # 784
PPIX = HPAD * WPAD            # 900
SPP = 912                     # padded plane stride (16B-aligned > PPIX)
NCHUNK = 2 * BPC              # 8 chunks of (sample, half-image)
HHALF = H // 2                # 14
CHUNK = HHALF * W             # 392 dense output elems per chunk
RUN = HHALF * WPAD            # 420: rhs run length / psum width per chunk
S1RUN = SPP // 2              # 456: ones-matmul run (half plane)
BN_EPS = 1e-5
NPRM = 24
GUARD = 16                    # fp8 guard elems around activation tiles
NTOT = float(N_CORES * NCHUNK * CHUNK)   # global BN count 25088
F32 = mybir.dt.float32
FP8 = mybir.dt.float8e4
AF = mybir.ActivationFunctionType
ALU = mybir.AluOpType
DR = mybir.MatmulPerfMode.DoubleRow


def _evac(nc, sc, ps, s1, sums, sumsqs, cv, prm, pcol, ci, oblk):
    """z = q[o]*S1 + BB from PSUM (strided: skip pad cols).  Stats come for
    free: the STT accumulates sum(z) on DVE; a Square pass on the otherwise
    idle ScalarE accumulates sum(z^2)."""
    psv = ps[:].rearrange("p (h w) -> p h w", h=HHALF)[:, :, 1:1 + W]
    s1v = s1[:].rearrange("p (h w) -> p h w", h=H)[
        :, (ci % 2) * HHALF:(ci % 2) * HHALF + HHALF, :]
    cvc = cv[oblk][:, ci * CHUNK:(ci + 1) * CHUNK]
    nc.vector.scalar_tensor_tensor(
        cvc.rearrange("p (h w) -> p h w", h=HHALF), s1v,
        prm[:, pcol['q'] + oblk:pcol['q'] + oblk + 1], psv,
        op0=ALU.mult, op1=ALU.add,
        accum_out=sums[:, oblk * NCHUNK + ci:oblk * NCHUNK + ci + 1])
    sqj = sc.tile([128, CHUNK], F32, tag="sqj", name="sqj", bufs=2)
    nc.scalar.activation(
        sqj[:], cvc, AF.Square,
        accum_out=sumsqs[:, oblk * NCHUNK + ci:oblk * NCHUNK + ci + 1])


def _s1_sample(nc, sc, psum, ones3, rhs420, rnd):
    """3x3 box of the channel sum for one sample -> s1 tile [128, H*W].
    The kh taps fold into PSUM accumulation of 3 shifted ones-matmuls per
    half; the kw taps are 2 shifted adds (GpSimd + DVE)."""
    hs = sc.tile([128, 2 * RUN], F32, tag="hs", name="hs", bufs=2)
    for half in range(2):
        h0 = half * HHALF
        ps2 = psum.tile([128, RUN], F32, tag="ps2", name=f"ps2_{rnd}",
                        bufs=2)
        for kh in range(3):
            nc.tensor.matmul(ps2[:], ones3, rhs420((h0 + kh) * WPAD),
                             start=(kh == 0), stop=(kh == 2), perf_mode=DR)
        nc.scalar.copy(hs[:, half * RUN:half * RUN + RUN], ps2[:])
    hsv = hs[:].rearrange("p (h w) -> p h w", h=H)
    w3 = sc.tile([128, H * W], F32, tag="w3", name="w3", bufs=2)
    w3v = w3[:].rearrange("p (h w) -> p h w", h=H)
    nc.gpsimd.tensor_add(w3v, hsv[:, :, 0:W], hsv[:, :, 1:1 + W])
    s1 = sc.tile([128, H * W], F32, tag="s1", name="s1", bufs=2)
    s1v = s1[:].rearrange("p (h w) -> p h w", h=H)
    nc.vector.tensor_add(s1v, w3v, hsv[:, :, 2:2 + W])
    return s1


def _main_sample(nc, sc, psum, rhs_ap, wv, b, rnd, s1, sums, sumsqs, cv,
                 prm, pcol):
    """All main conv matmuls + evac for sample b (2 halves x 2 blocks)."""
    for half in range(2):
        h0 = half * HHALF
        ci = 2 * b + half
        for oblk in range(NBLK):
            ps = psum.tile([128, RUN], F32, tag="ps", name=f"ps{rnd}",
                           bufs=6)
            for k9 in range(9):
                kh, kw = k9 // 3, k9 % 3
                nc.tensor.matmul(
                    ps[:], wv[:, k9, :, oblk * 128:(oblk + 1) * 128],
                    rhs_ap((h0 + kh) * WPAD + kw - 1),
                    start=(k9 == 0), stop=(k9 == 8), perf_mode=DR)
            _evac(nc, sc, ps, s1, sums, sumsqs, cv, prm, pcol, ci, oblk)


def _stats(nc, pools, rnd, sums, sumsqs, prm, pcol):
    """(sum, sumsq) per chunk -> per-core [128, (blk j)] -> one AllGather ->
    global scale/shift (128, NBLK) tiles indexed by oblk."""
    sbuf, psum, sc, dram = pools
    tr = sbuf.tile([128, 2 * NBLK], F32, name=f"tr{rnd}")
    for oblk in range(NBLK):
        nc.vector.reduce_sum(tr[:, oblk * 2:oblk * 2 + 1],
                             sums[:, oblk * NCHUNK:(oblk + 1) * NCHUNK],
                             axis=mybir.AxisListType.X)
        nc.vector.reduce_sum(tr[:, oblk * 2 + 1:oblk * 2 + 2],
                             sumsqs[:, oblk * NCHUNK:(oblk + 1) * NCHUNK],
                             axis=mybir.AxisListType.X)
    b_d = dram.tile([128, 2 * NBLK], F32, name=f"bd{rnd}")
    g_d = dram.tile([N_CORES * 128, 2 * NBLK], F32, name=f"gd{rnd}")
    nc.sync.dma_start(b_d[:], tr[:])
    nc.gpsimd.collective_compute(
        "AllGather", ALU.bypass,
        replica_groups=[list(range(N_CORES))],
        ins=[b_d.opt()], outs=[g_d.opt()])
    gst = sbuf.tile([128, N_CORES * 2 * NBLK], F32, name=f"gst{rnd}")
    nc.sync.dma_start(
        gst[:].rearrange("p (r j) -> p r j", r=N_CORES),
        g_d[:].rearrange("(r p) j -> p r j", r=N_CORES))
    gv = gst[:].rearrange("p (r j) -> p r j", r=N_CORES)

    mean = sbuf.tile([128, NBLK], F32, name=f"mean{rnd}")
    qn = sbuf.tile([128, NBLK], F32, name=f"qn{rnd}")
    msq = sbuf.tile([128, NBLK], F32, name=f"msq{rnd}")
    var = sbuf.tile([128, NBLK], F32, name=f"var{rnd}")
    sd = sbuf.tile([128, NBLK], F32, name=f"sd{rnd}")
    inv = sbuf.tile([128, NBLK], F32, name=f"inv{rnd}")
    scale = sbuf.tile([128, NBLK], F32, name=f"scale{rnd}")
    tmp = sbuf.tile([128, NBLK], F32, name=f"tmp{rnd}")
    shift = sbuf.tile([128, NBLK], F32, name=f"shift{rnd}")
    for oblk in range(NBLK):
        nc.vector.reduce_sum(mean[:, oblk:oblk + 1], gv[:, :, oblk * 2],
                             axis=mybir.AxisListType.X)
        nc.vector.reduce_sum(qn[:, oblk:oblk + 1], gv[:, :, oblk * 2 + 1],
                             axis=mybir.AxisListType.X)
    nc.vector.tensor_scalar_mul(mean[:], mean[:], 1.0 / NTOT)
    nc.vector.tensor_scalar_mul(qn[:], qn[:], 1.0 / NTOT)
    nc.vector.tensor_mul(msq[:], mean[:], mean[:])
    nc.vector.tensor_sub(var[:], qn[:], msq[:])
    for oblk in range(NBLK):
        nc.scalar.activation(sd[:, oblk:oblk + 1], var[:, oblk:oblk + 1],
                             AF.Sqrt,
                             bias=prm[:, pcol['eps'] + oblk:
                                      pcol['eps'] + oblk + 1],
                             scale=1.0)
    nc.vector.reciprocal(inv[:], sd[:])
    nc.vector.tensor_mul(scale[:], inv[:],
                         prm[:, pcol['g']:pcol['g'] + NBLK])
    nc.vector.tensor_mul(tmp[:], mean[:], scale[:])
    nc.vector.tensor_sub(shift[:], prm[:, pcol['b']:pcol['b'] + NBLK],
                         tmp[:])
    return scale, shift


def _apply_front(nc, sc, b, oblk, cv, xres, scale, shift, prm, pcol,
                 a2v=None):
    """BN-apply + residual + maxout coef for one (sample, block) span.
    Writes sign into a2v (next conv's fp8 input); returns (t, coef)."""
    cvs = cv[oblk][:, b * PIX:(b + 1) * PIX]
    xrs = xres[oblk][:, b * PIX:(b + 1) * PIX]
    u = sc.tile([128, PIX], F32, tag="u", name="u", bufs=2)
    nc.vector.tensor_scalar(u[:], cvs,
                            scale[:, oblk:oblk + 1],
                            shift[:, oblk:oblk + 1],
                            op0=ALU.mult, op1=ALU.add)
    t = sc.tile([128, PIX], F32, tag="t", name="t", bufs=4)
    nc.vector.tensor_add(t[:], u[:], xrs)
    tv = t[:].rearrange("p (h w) -> p h w", h=H)
    coef = sc.tile([128, PIX], F32, tag="coef", name="coef", bufs=4)
    if a2v is not None:
        # sign -> next conv's fp8 input (padded); coef read from it
        sg = a2v[b][:, oblk, 1:1 + H, 1:1 + W]
        nc.scalar.activation(sg, tv, AF.Sign)
        nc.scalar.activation(
            coef[:].rearrange("p (h w) -> p h w", h=H),
            sg, AF.Identity,
            bias=prm[:, pcol['hs'] + oblk:pcol['hs'] + oblk + 1],
            scale=prm[:, pcol['hp'] + oblk:pcol['hp'] + oblk + 1])
    else:
        sgt = sc.tile([128, PIX], F32, tag="sg", name="sg", bufs=2)
        nc.scalar.activation(sgt[:], t[:], AF.Sign)
        nc.scalar.activation(
            coef[:], sgt[:], AF.Identity,
            bias=prm[:, pcol['hs'] + oblk:pcol['hs'] + oblk + 1],
            scale=prm[:, pcol['hp'] + oblk:pcol['hp'] + oblk + 1])
    return t, coef


def _apply_back(nc, sc, b, oblk, t, coef, xres, ov=None, eng=None):
    """maxout multiply: in-place into xres (mid rounds) or out DMA."""
    e = eng if eng is not None else nc.vector
    xrs = xres[oblk][:, b * PIX:(b + 1) * PIX]
    if ov is None:
        e.tensor_mul(xrs, t[:], coef[:])
    else:
        och = sc.tile([128, PIX], F32, tag="och", name="och", bufs=4)
        e.tensor_mul(och[:], t[:], coef[:])
        q = nc.sync if (2 * b + oblk) % 2 == 0 else nc.scalar
        q.dma_start(ov[oblk * 128:oblk * 128 + 128, b], och[:])


def _apply_all(nc, sc, cv, xres, scale, shift, prm, pcol, a2v=None, ov=None):
    """All 8 apply spans with the maxout mul deferred one span, so the
    vector engine never blocks on the scalar sign->coef chain; muls
    alternate between DVE and GpSimd."""
    spans = [(b, oblk) for b in range(BPC) for oblk in range(NBLK)]
    pend = []
    n = 0
    for b, oblk in spans:
        t, coef = _apply_front(nc, sc, b, oblk, cv, xres, scale, shift,
                               prm, pcol, a2v=a2v)
        pend.append((b, oblk, t, coef))
        if len(pend) > 1:
            pb, po, pt, pc = pend.pop(0)
            _apply_back(nc, sc, pb, po, pt, pc, xres, ov=ov,
                        eng=nc.gpsimd if n % 2 else nc.vector)
            n += 1
    for pb, po, pt, pc in pend:
        _apply_back(nc, sc, pb, po, pt, pc, xres, ov=ov,
                    eng=nc.gpsimd if n % 2 else nc.vector)
        n += 1


def build():
    nc = bacc.Bacc("TRN2", target_bir_lowering=False, debug=False,
                   enable_asserts=True, num_devices=N_CORES)
    x_d = nc.dram_tensor("x", [BPC, C, H, W], F32, kind="ExternalInput")
    w1_d = nc.dram_tensor("w1t", [9, NBLK, 128, 256], FP8,
                          kind="ExternalInput")
    w2_d = nc.dram_tensor("w2t", [9, NBLK, 128, 256], FP8,
                          kind="ExternalInput")
    prm_d = nc.dram_tensor("prm", [128, NPRM], F32, kind="ExternalInput")
    out_d = nc.dram_tensor("out", [BPC, C, H, W], F32, kind="ExternalOutput")

    with tile.TileContext(nc) as tc:
        with (
            tc.tile_pool(name="sbuf", bufs=1) as sbuf,
            tc.tile_pool(name="psum", bufs=6, space="PSUM") as psum,
            tc.tile_pool(name="sc", bufs=2) as sc,
            tc.tile_pool(name="dram", bufs=1, space="DRAM") as dram,
        ):
            pools = (sbuf, psum, sc, dram)
            w1sb = sbuf.tile([128, 9 * NBLK * 256], FP8, name="w1sb")
            w2sb = sbuf.tile([128, 9 * NBLK * 256], FP8, name="w2sb")
            prm = sbuf.tile([128, NPRM], F32, name="prm")
            onesb = sbuf.tile([128, 256], FP8, name="onesb")
            xres = [sbuf.tile([128, BPC * PIX], F32, name=f"xres{i}")
                    for i in range(NBLK)]
            # per-sample padded activation tiles (two K-half planes each) so
            # subtile deps stay exact: conv(b) only waits on sign(b)
            ATW = GUARD + 2 * SPP + GUARD
            a1s = [sbuf.tile([128, ATW], FP8, name=f"a1s{b}")
                   for b in range(BPC)]
            a2s = [sbuf.tile([128, ATW], FP8, name=f"a2s{b}")
                   for b in range(BPC)]
            cv = [sbuf.tile([128, BPC * PIX], F32, name=f"cv{i}")
                  for i in range(NBLK)]

            # warmup collective, triggered first: absorbs the per-launch CC
            # spin-up behind the NRT init barrier so the stats AllGather
            # runs warm and starts promptly
            wu_i = dram.tile([1, 16], F32, name="wu_i")
            wu_o = dram.tile([N_CORES, 16], F32, name="wu_o")
            wu_s = sbuf.tile([1, 16], F32, name="wu_s")
            nc.gpsimd.memset(wu_s[:], 0.0)
            nc.gpsimd.dma_start(wu_i[:], wu_s[:])
            nc.gpsimd.collective_compute(
                "AllGather", ALU.bypass,
                replica_groups=[list(range(N_CORES))],
                ins=[wu_i.opt()], outs=[wu_o.opt()])
            nc.vector.memset(onesb[:], 1.0)
            for b in range(BPC):
                nc.vector.memset(a1s[b][:].bitcast(mybir.dt.uint32), 0)
                nc.gpsimd.memset(a2s[b][:].bitcast(mybir.dt.uint32), 0)
            xv = x_d[:].rearrange("b c h w -> c b (h w)")
            w1v = w1sb[:].rearrange("p (k i o) -> p k i o", k=9, i=NBLK)
            w1dv = w1_d[:].rearrange("k i p o -> p k i o")
            # queue layout: weights + prm first (small), then x in sample
            # order striped over both HWDGE queues, then w2
            nc.sync.dma_start(prm[:], prm_d[:])
            nc.scalar.dma_start(w1v[:, 0:3], w1dv[:, 0:3])
            nc.gpsimd.dma_start(w1v[:, 3:6], w1dv[:, 3:6])
            nc.sync.dma_start(w1v[:, 6:9], w1dv[:, 6:9])
            for b in range(BPC):
                nc.scalar.dma_start(xres[0][:, b * PIX:(b + 1) * PIX],
                                    xv[0:128, b])
                nc.sync.dma_start(xres[1][:, b * PIX:(b + 1) * PIX],
                                  xv[128:256, b])
            nc.sync.dma_start(
                w2sb[:].rearrange("p (k i o) -> p k i o", k=9, i=NBLK),
                w2_d[:].rearrange("k i p o -> p k i o"))

            def mk_rhs(atile, b, n):
                trun = atile[b][:, 0:2 * SPP].rearrange(
                    "p (i n) -> p i n", i=2)[:, :, 0:n]

                def rhs_ap(off):
                    return bass.AP(trun.tensor, GUARD + off, trun.ap)
                return rhs_ap

            rhs1 = [mk_rhs(a1s, b, RUN) for b in range(BPC)]
            rhs2 = [mk_rhs(a2s, b, RUN) for b in range(BPC)]
            ones3 = onesb[:].rearrange("p (i o) -> p i o", i=2)
            wv1 = w1sb[:].rearrange("p (k i o) -> p k i o", k=9, i=NBLK)
            wv2 = w2sb[:].rearrange("p (k i o) -> p k i o", k=9, i=NBLK)

            def aview(atile, b):
                return atile[b][:, GUARD:GUARD + 2 * SPP].rearrange(
                    "p (i n) -> p i n", i=2)[:, :, 0:PPIX].rearrange(
                    "p i (h w) -> p i h w", h=HPAD)

            a1v = [aview(a1s, b) for b in range(BPC)]
            a2v = [aview(a2s, b) for b in range(BPC)]
            xrvs = [xres[i][:].rearrange("p (b h w) -> p b h w", b=BPC, h=H)
                    for i in range(NBLK)]
            ov = out_d[:].rearrange("b c h w -> c b (h w)")

            pcol1 = {'g': 0, 'b': 2, 'hp': 4, 'hs': 6, 'eps': 16, 'q': 20}
            pcol2 = {'g': 8, 'b': 10, 'hp': 12, 'hs': 14, 'eps': 18, 'q': 22}

            def signs(b):
                for i in range(NBLK):
                    nc.scalar.activation(a1v[b][:, i, 1:1 + H, 1:1 + W],
                                         xrvs[i][:, b], AF.Sign)

            # ---- round 1: per-sample sign -> S1 -> main conv; the next
            # sample's sign is issued between S1 and main so ScalarE runs it
            # before this sample's evac Squares ----
            sums1 = sbuf.tile([128, NCHUNK * NBLK], F32, name="sums1")
            sumsqs1 = sbuf.tile([128, NCHUNK * NBLK], F32, name="sumsqs1")
            signs(0)
            for b in range(BPC):
                s1 = _s1_sample(nc, sc, psum, ones3, rhs1[b], 1)
                if b + 1 < BPC:
                    signs(b + 1)
                _main_sample(nc, sc, psum, rhs1[b], wv1, b, 1, s1,
                             sums1, sumsqs1, cv, prm, pcol1)
            scale1, shift1 = _stats(nc, pools, 1, sums1, sumsqs1, prm, pcol1)

            # ---- apply round 1 interleaved per sample with round 2 conv;
            # the maxout muls go to GpSimd and are deferred one span ----
            sums2 = sbuf.tile([128, NCHUNK * NBLK], F32, name="sums2")
            sumsqs2 = sbuf.tile([128, NCHUNK * NBLK], F32, name="sumsqs2")
            pend = []
            for b in range(BPC):
                for oblk in range(NBLK):
                    t, coef = _apply_front(nc, sc, b, oblk, cv, xres,
                                           scale1, shift1, prm, pcol1,
                                           a2v=a2v)
                    pend.append((b, oblk, t, coef))
                    if len(pend) > 1:
                        pb, po, pt, pc = pend.pop(0)
                        _apply_back(nc, sc, pb, po, pt, pc, xres,
                                    eng=nc.gpsimd)
                s1 = _s1_sample(nc, sc, psum, ones3, rhs2[b], 2)
                _main_sample(nc, sc, psum, rhs2[b], wv2, b, 2, s1,
                             sums2, sumsqs2, cv, prm, pcol2)
            scale2, shift2 = _stats(nc, pools, 2, sums2, sumsqs2, prm, pcol2)
            for pb, po, pt, pc in pend:
                _apply_back(nc, sc, pb, po, pt, pc, xres, eng=nc.vector)

            # ---- final apply + output DMA ----
            _apply_all(nc, sc, cv, xres, scale2, shift2, prm, pcol2, ov=ov)

    nc.compile()
    return nc


def _prep_weight(w):
    """(O,I,3,3) fp32 -> sign lhsT (9, iblk, 128, 256) fp8 (+-1, exact),
    plus per-output-channel alpha, beta (float64)."""
    w = w.astype(np.float64)
    beta = w.mean(axis=(1, 2, 3))
    alpha = np.sqrt(((w - beta[:, None, None, None]) ** 2)
                    .mean(axis=(1, 2, 3)))
    s = np.sign(w - beta[:, None, None, None]).astype(np.float32)
    wt = s.transpose(2, 3, 1, 0).reshape(9, C, C)   # (k9, i, o)
    wt = wt.reshape(9, NBLK, 128, C)                # (k9, iblk, i, o)
    return wt.astype(ml_dtypes.float8_e4m3), alpha, beta


def make_in_maps(inputs):
    x = np.asarray(inputs['x'], np.float32)
    aa1 = float(np.asarray(inputs['alpha_a1']).reshape(-1)[0])
    aa2 = float(np.asarray(inputs['alpha_a2']).reshape(-1)[0])
    w1t, al1, be1 = _prep_weight(np.asarray(inputs['w1'], np.float32))
    w2t, al2, be2 = _prep_weight(np.asarray(inputs['w2'], np.float32))
    prm = np.zeros((128, NPRM), np.float32)
    f1 = 1.0 / (aa1 * al1)      # z scale relative to the true conv output
    f2 = 1.0 / (aa2 * al2)
    p1 = np.asarray(inputs['pos1'], np.float64)
    n1 = np.asarray(inputs['neg1'], np.float64)
    p2 = np.asarray(inputs['pos2'], np.float64)
    n2 = np.asarray(inputs['neg2'], np.float64)
    cols = ((0, np.asarray(inputs['g1'], np.float64)),
            (2, np.asarray(inputs['b1'], np.float64)),
            (4, (p1 - n1) / 2),
            (6, (p1 + n1) / 2),
            (8, np.asarray(inputs['g2'], np.float64)),
            (10, np.asarray(inputs['b2'], np.float64)),
            (12, (p2 - n2) / 2),
            (14, (p2 + n2) / 2),
            (16, BN_EPS * f1 * f1),
            (18, BN_EPS * f2 * f2),
            (20, be1 / al1),
            (22, be2 / al2))
    for base, arr in cols:
        prm[:, base] = arr[:128]
        prm[:, base + 1] = arr[128:]
    in_maps = []
    for c in range(N_CORES):
        in_maps.append({
            'x': np.ascontiguousarray(x[c * BPC:(c + 1) * BPC]),
            'w1t': w1t, 'w2t': w2t, 'prm': prm,
        })
    return in_maps


_CACHE = {}


def kernel(**inputs):
    in_maps = make_in_maps(inputs)
    if 'run' not in _CACHE:
        nc = build()
        _CACHE['nc'] = nc
        _CACHE['run'] = _make_runner(nc)
    outs = _CACHE['run'](in_maps)
    return np.concatenate([outs[c] for c in range(N_CORES)], axis=0)


def _make_runner(nc):
    """Build a cached PJRT executable (same path run_bass_kernel_spmd takes
    under axon, via bass2jax) so repeat calls don't re-trace."""
    import jax
    import jax.numpy as jnp
    from jax.sharding import Mesh, PartitionSpec
    from jax.experimental.shard_map import shard_map
    from concourse import bass2jax

    bass2jax.install_neuronx_cc_hook()
    partition_name = (nc.partition_id_tensor.name
                      if nc.partition_id_tensor else None)
    in_names = []
    out_names = []
    out_avals = []
    for alloc in nc.m.functions[0].allocations:
        if not isinstance(alloc, mybir.MemoryLocationSet):
            continue
        name = alloc.memorylocations[0].name
        if alloc.kind == "ExternalInput":
            if name != partition_name:
                in_names.append(name)
        elif alloc.kind == "ExternalOutput":
            shape = tuple(alloc.tensor_shape)
            dtype = mybir.dt.np(alloc.dtype)
            out_names.append(name)
            out_avals.append(jax.core.ShapedArray(shape, dtype))
    n_params = len(in_names)
    all_names = in_names + out_names
    if partition_name is not None:
        all_names = all_names + [partition_name]

    def _body(*args):
        operands = list(args)
        if partition_name is not None:
            operands.append(bass2jax.partition_id_tensor())
        outs = bass2jax._bass_exec_p.bind(
            *operands,
            out_avals=tuple(out_avals),
            in_names=tuple(all_names),
            out_names=tuple(out_names),
            lowering_input_output_aliases=(),
            sim_require_finite=True,
            sim_require_nnan=True,
            nc=nc,
        )
        return tuple(outs)

    devices = jax.devices()[:N_CORES]
    mesh = Mesh(np.asarray(devices), ("core",))
    n_outs = len(out_names)
    sharded = jax.jit(
        shard_map(_body, mesh=mesh,
                  in_specs=(PartitionSpec("core"),) * (n_params + n_outs),
                  out_specs=(PartitionSpec("core"),) * n_outs,
                  check_rep=False),
        donate_argnums=tuple(range(n_params, n_params + n_outs)),
        keep_unused=True,
    )
    sharded_nodonate = jax.jit(
        shard_map(_body, mesh=mesh,
                  in_specs=(PartitionSpec("core"),) * (n_params + n_outs),
                  out_specs=(PartitionSpec("core"),) * n_outs,
                  check_rep=False),
        keep_unused=True,
    )

    def run(in_maps):
        concat_in = [
            np.concatenate([np.asarray(in_maps[c][n]) for c in range(N_CORES)],
                           axis=0)
            for n in in_names
        ]
        concat_zeros = [
            np.zeros((N_CORES * a.shape[0], *a.shape[1:]), a.dtype)
            for a in out_avals
        ]
        out_arrs = sharded(*concat_in, *concat_zeros)
        i = out_names.index("out")
        full = np.asarray(out_arrs[i]).reshape(N_CORES, *out_avals[i].shape)
        return [full[c] for c in range(N_CORES)]

    def stage(in_maps):
        """device_put inputs once; return a dispatch closure for timing."""
        from jax.sharding import NamedSharding
        sh = NamedSharding(mesh, PartitionSpec("core"))
        concat_in = [
            jax.device_put(np.concatenate(
                [np.asarray(in_maps[c][n]) for c in range(N_CORES)], axis=0), sh)
            for n in in_names
        ]
        concat_zeros = [
            jax.device_put(
                np.zeros((N_CORES * a.shape[0], *a.shape[1:]), a.dtype), sh)
            for a in out_avals
        ]

        def dispatch():
            return sharded_nodonate(*concat_in, *concat_zeros)

        return dispatch

    run.stage = stage
    return run
